# revision 23
# baseline (speedup 1.0000x reference)
"""Masked attention (B=2,H=8,S=4096,D=64) on 8 Trainium2 NeuronCores.

Strategy (head-parallel, no collectives): the 16 (b,h) pairs shard 2-per-core.
The attention matrix (the reference's second output, [B,H,S,S] f32 ~= 1 GiB) is
what makes this memory-bound, so the device computes and ships everything in
the TRANSPOSED orientation S^T[k,q] = K @ Q^T:

  - S^T tile  = matmul(lhsT=K^T[d,k128], rhs=Q^T[d,q512])   (Q pre-scaled 1/8)
  - E^T       = exp(S^T)      on ScalarE (PSUM -> SBUF bf16, one pass)
  - Em^T      = E^T * mask^T  on VectorE (bf16 2x mode)     -> DMA out (bf16)
  - O_aug^T  += V_aug^T-style matmul(lhsT=V_aug[k128,65], rhs=Em^T[k128,q512]),
                where V_aug's 65th column of ones makes row 64 the softmax
                denominators r[q] for free.

The host then normalizes (divide by r) and transposes back - a pure scaling
epilogue; masked entries are exactly 0 because exp(-1e9) == 0 == exp(s)*0.
bf16 storage of the unnormalized exp keeps rel-err ~5e-3 while halving the
dominant DMA traffic.
"""

import os
import sys

import numpy as np
import ml_dtypes

B, H, S, D = 2, 8, 4096, 64
NPAIRS = 2            # (b*h) pairs per core
N_CORES = 8
QB = 512              # q block (matmul free dim)
NQB = S // QB         # 8 q blocks
KT = 128              # k tile (contraction rows per S^T tile)
NKT = S // KT         # 32 k tiles
CH = 2                # k tiles per PSUM chunk (exp/mask granularity)
NCH = NKT // CH       # 16 chunks per strip
VA = D + 1            # V columns + ones column

BF16 = ml_dtypes.bfloat16

_cache = {}
last_exec_time_ns = None
last_profile = None


def _install_axon_ntff_hook():
    """Provide the missing antenv.axon_hooks module (ctypes NTFF profiling)."""
    import contextlib
    import ctypes
    import types

    if "antenv.axon_hooks" in sys.modules:
        return
    so_path = "/opt/axon/libaxon_pjrt.so"
    try:
        lib = ctypes.CDLL(so_path)
        lib.axon_start_nrt_profile.argtypes = [
            ctypes.POINTER(ctypes.c_int64),
            ctypes.c_size_t,
        ]
        lib.axon_start_nrt_profile.restype = ctypes.c_int64
        lib.axon_stop_nrt_profile.argtypes = [ctypes.c_char_p]
        lib.axon_stop_nrt_profile.restype = ctypes.c_int64
    except (OSError, AttributeError):
        return

    @contextlib.contextmanager
    def _hook(output_dir, device_ids=None):
        import jax

        jax.devices()
        rc = lib.axon_start_nrt_profile(None, 0)
        if rc != 0:
            raise RuntimeError(f"axon_start_nrt_profile rc={rc}")
        try:
            yield
        finally:
            n = lib.axon_stop_nrt_profile(str(output_dir).encode())
            print(f"ntff profile: {n} file(s) -> {output_dir}", file=sys.stderr)

    mod = types.ModuleType("antenv.axon_hooks")
    mod.get_axon_ntff_profile_hook = lambda: _hook
    mod.set_axon_ntff_profile_hook = lambda h: None
    import antenv

    antenv.axon_hooks = mod
    sys.modules["antenv.axon_hooks"] = mod

    from concourse import bass_utils

    bass_utils.upload_artifacts = lambda tmpdir: f"file://{tmpdir}"


def _build():
    import concourse.tile as tile
    from concourse import bacc, mybir

    bf = mybir.dt.bfloat16
    f32 = mybir.dt.float32

    nc = bacc.Bacc("TRN2", target_bir_lowering=False, debug=False,
                   num_devices=N_CORES)

    # qt: Q^T duplicated into both row-halves [128, S] so the two row-packed
    # S^T matmuls (row groups 0-63 / 64-127) each stream their own rhs copy.
    # kt: K^T packed pairwise: partitions 0-63 hold even k-tiles, 64-127 odd.
    qt = nc.declare_dram_parameter("qt", [NPAIRS, 2 * D, S], bf, isOutput=False)
    kt = nc.declare_dram_parameter("kt", [NPAIRS, 2 * D, S // 2], bf, isOutput=False)
    va = nc.declare_dram_parameter("va", [NPAIRS, KT, NKT, VA], bf, isOutput=False)
    mt = nc.declare_dram_parameter("mt", [NKT, KT, S], mybir.dt.uint8,
                                   isOutput=False)
    eout = nc.declare_dram_parameter("eout", [NPAIRS, NKT, KT, S], bf, isOutput=True)
    oaug = nc.declare_dram_parameter("oaug", [NPAIRS, VA, S], f32, isOutput=True)

    mt_r = mt.rearrange("t p q -> p t q")
    eout_r = eout.rearrange("v t p q -> v p t q")

    with tile.TileContext(nc) as tc:
        with (
            tc.tile_pool(name="const", bufs=1) as cpool,
            tc.tile_pool(name="mask8", bufs=2) as m8pool,
            tc.tile_pool(name="mask", bufs=2) as mpool,
            tc.tile_pool(name="spsum", bufs=3, space="PSUM") as spool,
            tc.tile_pool(name="opsum", bufs=2, space="PSUM") as opool,
            tc.tile_pool(name="echunk", bufs=6) as epool,
            tc.tile_pool(name="emchunk", bufs=20) as empool,
            tc.tile_pool(name="osb", bufs=2) as ospool,
        ):
            qt_sb, kt_sb, va_sb = [], [], []
            for p in range(NPAIRS):
                qts = cpool.tile([2 * D, S], bf, tag=f"qt{p}")
                nc.sync.dma_start(out=qts[:], in_=qt[p])
                qt_sb.append(qts)
                kts = cpool.tile([2 * D, S // 2], bf, tag=f"kt{p}")
                nc.sync.dma_start(out=kts[:], in_=kt[p])
                kt_sb.append(kts)
                vas = cpool.tile([KT, NKT, VA], bf, tag=f"va{p}")
                nc.sync.dma_start(out=vas[:], in_=va[p])
                va_sb.append(vas)

            out_engines = [nc.sync, nc.scalar]
            for qb in range(NQB):
                qsl = slice(qb * QB, (qb + 1) * QB)
                m8_strip = m8pool.tile([KT, NKT, QB], mybir.dt.uint8, tag="m8")
                nc.sync.dma_start(out=m8_strip[:], in_=mt_r[:, :, qsl])
                # u8 -> bf16 cast on the otherwise-idle GpSimd engine.
                m_strip = mpool.tile([KT, NKT, QB], bf, tag="mstrip")
                for h in range(4):
                    nc.gpsimd.tensor_copy(
                        m_strip[:, h * 8:(h + 1) * 8, :],
                        m8_strip[:, h * 8:(h + 1) * 8, :],
                    )

                for p in range(NPAIRS):
                    o_ps = opool.tile([VA, QB], f32, tag="ops")
                    for c in range(NCH):
                        ksl = slice(c * KT, (c + 1) * KT)
                        s_ps = spool.tile([KT, CH, QB], f32, tag="sps")
                        # Two k-tiles computed concurrently in PE row groups
                        # 0-63 / 64-127 (contraction is only D=64).
                        nc.tensor.matmul(
                            s_ps[:, 0, :],
                            lhsT=kt_sb[p][0:D, ksl],
                            rhs=qt_sb[p][0:D, qsl],
                            start=True, stop=True,
                            tile_position=(0, 0),
                        )
                        nc.tensor.matmul(
                            s_ps[:, 1, :],
                            lhsT=kt_sb[p][D:2 * D, ksl],
                            rhs=qt_sb[p][D:2 * D, qsl],
                            start=True, stop=True,
                            tile_position=(64, 0),
                        )
                        e_ch = epool.tile([KT, CH, QB], bf, tag="ech")
                        nc.scalar.activation(
                            e_ch[:], s_ps[:],
                            mybir.ActivationFunctionType.Exp,
                        )
                        em_ch = empool.tile([KT, CH, QB], bf, tag="emch")
                        nc.vector.tensor_tensor(
                            em_ch[:], e_ch[:],
                            m_strip[:, c * CH:(c + 1) * CH, :],
                            mybir.AluOpType.mult,
                        )
                        for j in range(CH):
                            k_tile = c * CH + j
                            nc.tensor.matmul(
                                o_ps[:],
                                lhsT=va_sb[p][:, k_tile, :],
                                rhs=em_ch[:, j, :],
                                start=(k_tile == 0),
                                stop=(k_tile == NKT - 1),
                            )
                        eng = out_engines[c % len(out_engines)]
                        eng.dma_start(
                            out=eout_r[p, :, c * CH:(c + 1) * CH, qsl],
                            in_=em_ch[:],
                        )
                    o_sb = ospool.tile([VA, QB], f32, tag="osb")
                    nc.vector.tensor_copy(o_sb[:], o_ps[:])
                    nc.sync.dma_start(out=oaug[p, :, qsl], in_=o_sb[:])

    nc.compile()
    return nc


def _get_nc():
    if "nc" not in _cache:
        _install_axon_ntff_hook()
        _cache["nc"] = _build()
    return _cache["nc"]


def kernel(query, key, value, mask):
    global last_exec_time_ns, last_profile
    from concourse.bass_utils import run_bass_kernel_spmd

    nc = _get_nc()

    q = np.asarray(query, dtype=np.float32).reshape(B * H, S, D)
    k = np.asarray(key, dtype=np.float32).reshape(B * H, S, D)
    v = np.asarray(value, dtype=np.float32).reshape(B * H, S, D)
    m = np.asarray(mask)

    # Host prep: transpose Q/K to [d, s] (fold the 1/sqrt(D) scale into Q),
    # append the ones column to V and pack partition-major, transpose mask.
    # Q^T is duplicated into rows 64-127 and K^T packed pairwise (even k-tiles
    # in rows 0-63, odd in 64-127) for the row-packed concurrent matmuls.
    qt_t = np.ascontiguousarray(
        (q * (1.0 / np.sqrt(D))).transpose(0, 2, 1)).astype(BF16)
    qt_full = np.concatenate([qt_t, qt_t], axis=1)            # [bh, 128, S]
    kt_t = k.transpose(0, 2, 1)                               # [bh, 64, S]
    kt_full = np.ascontiguousarray(
        kt_t.reshape(B * H, D, NKT // 2, 2, KT)
        .transpose(0, 3, 1, 2, 4)
        .reshape(B * H, 2 * D, S // 2)).astype(BF16)
    v_aug = np.concatenate(
        [v, np.ones((B * H, S, 1), np.float32)], axis=2)      # [bh, S, 65]
    va_full = np.ascontiguousarray(
        v_aug.reshape(B * H, NKT, KT, VA).transpose(0, 2, 1, 3)).astype(BF16)
    mt_host = np.ascontiguousarray(m.T).astype(np.uint8).reshape(NKT, KT, S)

    in_maps = []
    for c in range(N_CORES):
        sl = slice(NPAIRS * c, NPAIRS * (c + 1))
        in_maps.append({
            "qt": qt_full[sl],
            "kt": kt_full[sl],
            "va": va_full[sl],
            "mt": mt_host,
        })

    trace = bool(int(os.environ.get("BASS_KERNEL_TRACE", "0")))
    res = run_bass_kernel_spmd(nc, in_maps, core_ids=list(range(N_CORES)),
                               trace=trace)
    last_exec_time_ns = res.exec_time_ns
    last_profile = res.profile_json

    out = np.empty((B * H, S, D), np.float32)
    attn = np.empty((B * H, S, S), np.float32)
    for c in range(N_CORES):
        eo = res.results[c]["eout"]     # [NPAIRS, NKT, KT, S] bf16 (k-major)
        oa = res.results[c]["oaug"]     # [NPAIRS, VA, S] f32
        for i in range(NPAIRS):
            bh = NPAIRS * c + i
            r = oa[i, D, :].astype(np.float32)    # softmax denominators [q]
            r = np.where(r == 0.0, 1.0, r)
            inv_r = (1.0 / r).astype(np.float32)
            et = eo[i].reshape(S, S).astype(np.float32)       # [k, q]
            attn[bh] = (et * inv_r[None, :]).T
            out[bh] = (oa[i, :D, :] * inv_r[None, :]).T
    output = out.reshape(B, H, S, D)
    attention = attn.reshape(B, H, S, S)
    return (output, attention)


# revision 27
# speedup vs baseline: 2.0022x; 2.0022x over previous
"""Masked attention (B=2,H=8,S=4096,D=64) on 8 Trainium2 NeuronCores.

Strategy (head-parallel, no collectives): the 16 (b,h) pairs shard 2-per-core.
The attention matrix (the reference's second output, [B,H,S,S] f32 ~= 1 GiB) is
what makes this memory-bound, so the device computes and ships everything in
the TRANSPOSED orientation S^T[k,q] = K @ Q^T:

  - S^T tile  = matmul(lhsT=K^T[d,k128], rhs=Q^T[d,q512])   (Q pre-scaled 1/8)
  - E^T       = exp(S^T)      on ScalarE (PSUM -> SBUF bf16, one pass)
  - Em^T      = E^T * mask^T  on VectorE (bf16 2x mode)     -> DMA out (bf16)
  - O_aug^T  += V_aug^T-style matmul(lhsT=V_aug[k128,65], rhs=Em^T[k128,q512]),
                where V_aug's 65th column of ones makes row 64 the softmax
                denominators r[q] for free.

The host then normalizes (divide by r) and transposes back - a pure scaling
epilogue; masked entries are exactly 0 because exp(-1e9) == 0 == exp(s)*0.
bf16 storage of the unnormalized exp keeps rel-err ~5e-3 while halving the
dominant DMA traffic.
"""

import os
import sys

import numpy as np
import ml_dtypes

B, H, S, D = 2, 8, 4096, 64
NPAIRS = 2            # (b*h) pairs per core
N_CORES = 8
QB = 512              # q block (matmul free dim)
NQB = S // QB         # 8 q blocks
KT = 128              # k tile (contraction rows per S^T tile)
NKT = S // KT         # 32 k tiles
CH = 2                # k tiles per PSUM chunk (exp/mask granularity)
NCH = NKT // CH       # 16 chunks per strip
VA = D + 1            # V columns + ones column

BF16 = ml_dtypes.bfloat16

_cache = {}
last_exec_time_ns = None
last_profile = None


def _install_axon_ntff_hook():
    """Provide the missing antenv.axon_hooks module (ctypes NTFF profiling)."""
    import contextlib
    import ctypes
    import types

    if "antenv.axon_hooks" in sys.modules:
        return
    so_path = "/opt/axon/libaxon_pjrt.so"
    try:
        lib = ctypes.CDLL(so_path)
        lib.axon_start_nrt_profile.argtypes = [
            ctypes.POINTER(ctypes.c_int64),
            ctypes.c_size_t,
        ]
        lib.axon_start_nrt_profile.restype = ctypes.c_int64
        lib.axon_stop_nrt_profile.argtypes = [ctypes.c_char_p]
        lib.axon_stop_nrt_profile.restype = ctypes.c_int64
    except (OSError, AttributeError):
        return

    @contextlib.contextmanager
    def _hook(output_dir, device_ids=None):
        import jax

        jax.devices()
        rc = lib.axon_start_nrt_profile(None, 0)
        if rc != 0:
            raise RuntimeError(f"axon_start_nrt_profile rc={rc}")
        try:
            yield
        finally:
            n = lib.axon_stop_nrt_profile(str(output_dir).encode())
            print(f"ntff profile: {n} file(s) -> {output_dir}", file=sys.stderr)

    mod = types.ModuleType("antenv.axon_hooks")
    mod.get_axon_ntff_profile_hook = lambda: _hook
    mod.set_axon_ntff_profile_hook = lambda h: None
    import antenv

    antenv.axon_hooks = mod
    sys.modules["antenv.axon_hooks"] = mod

    from concourse import bass_utils

    bass_utils.upload_artifacts = lambda tmpdir: f"file://{tmpdir}"


def _build():
    import concourse.tile as tile
    from concourse import bacc, mybir

    bf = mybir.dt.bfloat16
    f32 = mybir.dt.float32

    nc = bacc.Bacc("TRN2", target_bir_lowering=False, debug=False,
                   num_devices=N_CORES)

    # qt: Q^T duplicated into both row-halves [128, S] so the two row-packed
    # S^T matmuls (row groups 0-63 / 64-127) each stream their own rhs copy.
    # kt: K^T packed pairwise: partitions 0-63 hold even k-tiles, 64-127 odd.
    qt = nc.declare_dram_parameter("qt", [NPAIRS, 2 * D, S], bf, isOutput=False)
    kt = nc.declare_dram_parameter("kt", [NPAIRS, 2 * D, S // 2], bf, isOutput=False)
    va = nc.declare_dram_parameter("va", [NPAIRS, KT, NKT, VA], bf, isOutput=False)
    mt = nc.declare_dram_parameter("mt", [NKT, KT, S], bf, isOutput=False)
    eout = nc.declare_dram_parameter("eout", [NPAIRS, NKT, KT, S], bf, isOutput=True)
    oaug = nc.declare_dram_parameter("oaug", [NPAIRS, VA, S], f32, isOutput=True)

    mt_r = mt.rearrange("t p q -> p t q")
    eout_r = eout.rearrange("v t p q -> v p t q")

    with tile.TileContext(nc) as tc:
        with (
            tc.tile_pool(name="const", bufs=1) as cpool,
            tc.tile_pool(name="mask", bufs=2) as mpool,
            tc.tile_pool(name="spsum", bufs=3, space="PSUM") as spool,
            tc.tile_pool(name="opsum", bufs=2, space="PSUM") as opool,
            tc.tile_pool(name="echunk", bufs=6) as epool,
            tc.tile_pool(name="emchunk", bufs=20) as empool,
            tc.tile_pool(name="osb", bufs=2) as ospool,
        ):
            qt_sb, kt_sb, va_sb = [], [], []
            for p in range(NPAIRS):
                qts = cpool.tile([2 * D, S], bf, tag=f"qt{p}")
                nc.sync.dma_start(out=qts[:], in_=qt[p])
                qt_sb.append(qts)
                kts = cpool.tile([2 * D, S // 2], bf, tag=f"kt{p}")
                nc.sync.dma_start(out=kts[:], in_=kt[p])
                kt_sb.append(kts)
                vas = cpool.tile([KT, NKT, VA], bf, tag=f"va{p}")
                nc.sync.dma_start(out=vas[:], in_=va[p])
                va_sb.append(vas)

            out_engines = [nc.sync, nc.scalar]
            for qb in range(NQB):
                qsl = slice(qb * QB, (qb + 1) * QB)
                m_strip = mpool.tile([KT, NKT, QB], bf, tag="mstrip")
                nc.sync.dma_start(out=m_strip[:], in_=mt_r[:, :, qsl])

                for p in range(NPAIRS):
                    o_ps = opool.tile([VA, QB], f32, tag="ops")
                    for c in range(NCH):
                        ksl = slice(c * KT, (c + 1) * KT)
                        s_ps = spool.tile([KT, CH, QB], f32, tag="sps")
                        # Two k-tiles computed concurrently in PE row groups
                        # 0-63 / 64-127 (contraction is only D=64).
                        nc.tensor.matmul(
                            s_ps[:, 0, :],
                            lhsT=kt_sb[p][0:D, ksl],
                            rhs=qt_sb[p][0:D, qsl],
                            start=True, stop=True,
                            tile_position=(0, 0),
                        )
                        nc.tensor.matmul(
                            s_ps[:, 1, :],
                            lhsT=kt_sb[p][D:2 * D, ksl],
                            rhs=qt_sb[p][D:2 * D, qsl],
                            start=True, stop=True,
                            tile_position=(64, 0),
                        )
                        e_ch = epool.tile([KT, CH, QB], bf, tag="ech")
                        nc.scalar.activation(
                            e_ch[:], s_ps[:],
                            mybir.ActivationFunctionType.Exp,
                        )
                        em_ch = empool.tile([KT, CH, QB], bf, tag="emch")
                        nc.vector.tensor_tensor(
                            em_ch[:], e_ch[:],
                            m_strip[:, c * CH:(c + 1) * CH, :],
                            mybir.AluOpType.mult,
                        )
                        for j in range(CH):
                            k_tile = c * CH + j
                            nc.tensor.matmul(
                                o_ps[:],
                                lhsT=va_sb[p][:, k_tile, :],
                                rhs=em_ch[:, j, :],
                                start=(k_tile == 0),
                                stop=(k_tile == NKT - 1),
                            )
                        eng = out_engines[c % len(out_engines)]
                        eng.dma_start(
                            out=eout_r[p, :, c * CH:(c + 1) * CH, qsl],
                            in_=em_ch[:],
                        )
                    o_sb = ospool.tile([VA, QB], f32, tag="osb")
                    nc.vector.tensor_copy(o_sb[:], o_ps[:])
                    nc.sync.dma_start(out=oaug[p, :, qsl], in_=o_sb[:])

    nc.compile()
    return nc


def _get_nc():
    if "nc" not in _cache:
        _install_axon_ntff_hook()
        _cache["nc"] = _build()
    return _cache["nc"]


def kernel(query, key, value, mask):
    global last_exec_time_ns, last_profile
    from concourse.bass_utils import run_bass_kernel_spmd

    nc = _get_nc()

    q = np.asarray(query, dtype=np.float32).reshape(B * H, S, D)
    k = np.asarray(key, dtype=np.float32).reshape(B * H, S, D)
    v = np.asarray(value, dtype=np.float32).reshape(B * H, S, D)
    m = np.asarray(mask)

    # Host prep: transpose Q/K to [d, s] (fold the 1/sqrt(D) scale into Q),
    # append the ones column to V and pack partition-major, transpose mask.
    # Q^T is duplicated into rows 64-127 and K^T packed pairwise (even k-tiles
    # in rows 0-63, odd in 64-127) for the row-packed concurrent matmuls.
    qt_t = np.ascontiguousarray(
        (q * (1.0 / np.sqrt(D))).transpose(0, 2, 1)).astype(BF16)
    qt_full = np.concatenate([qt_t, qt_t], axis=1)            # [bh, 128, S]
    kt_t = k.transpose(0, 2, 1)                               # [bh, 64, S]
    kt_full = np.ascontiguousarray(
        kt_t.reshape(B * H, D, NKT // 2, 2, KT)
        .transpose(0, 3, 1, 2, 4)
        .reshape(B * H, 2 * D, S // 2)).astype(BF16)
    v_aug = np.concatenate(
        [v, np.ones((B * H, S, 1), np.float32)], axis=2)      # [bh, S, 65]
    va_full = np.ascontiguousarray(
        v_aug.reshape(B * H, NKT, KT, VA).transpose(0, 2, 1, 3)).astype(BF16)
    mt_host = np.ascontiguousarray(m.T.astype(np.float32)).astype(BF16)
    mt_host = mt_host.reshape(NKT, KT, S)

    in_maps = []
    for c in range(N_CORES):
        sl = slice(NPAIRS * c, NPAIRS * (c + 1))
        in_maps.append({
            "qt": qt_full[sl],
            "kt": kt_full[sl],
            "va": va_full[sl],
            "mt": mt_host,
        })

    trace = bool(int(os.environ.get("BASS_KERNEL_TRACE", "0")))
    res = run_bass_kernel_spmd(nc, in_maps, core_ids=list(range(N_CORES)),
                               trace=trace)
    last_exec_time_ns = res.exec_time_ns
    last_profile = res.profile_json

    out = np.empty((B * H, S, D), np.float32)
    attn = np.empty((B * H, S, S), np.float32)
    for c in range(N_CORES):
        eo = res.results[c]["eout"]     # [NPAIRS, NKT, KT, S] bf16 (k-major)
        oa = res.results[c]["oaug"]     # [NPAIRS, VA, S] f32
        for i in range(NPAIRS):
            bh = NPAIRS * c + i
            r = oa[i, D, :].astype(np.float32)    # softmax denominators [q]
            r = np.where(r == 0.0, 1.0, r)
            inv_r = (1.0 / r).astype(np.float32)
            et = eo[i].reshape(S, S).astype(np.float32)       # [k, q]
            attn[bh] = (et * inv_r[None, :]).T
            out[bh] = (oa[i, :D, :] * inv_r[None, :]).T
    output = out.reshape(B, H, S, D)
    attention = attn.reshape(B, H, S, S)
    return (output, attention)


# revision 28
# speedup vs baseline: 2.0199x; 1.0089x over previous
"""Masked attention (B=2,H=8,S=4096,D=64) on 8 Trainium2 NeuronCores.

Strategy (head-parallel, no collectives): the 16 (b,h) pairs shard 2-per-core.
The attention matrix (the reference's second output, [B,H,S,S] f32 ~= 1 GiB) is
what makes this memory-bound, so the device computes and ships everything in
the TRANSPOSED orientation S^T[k,q] = K @ Q^T:

  - S^T tile  = matmul(lhsT=K^T[d,k128], rhs=Q^T[d,q512])   (Q pre-scaled 1/8)
  - E^T       = exp(S^T)      on ScalarE (PSUM -> SBUF bf16, one pass)
  - Em^T      = E^T * mask^T  on VectorE (bf16 2x mode)     -> DMA out (bf16)
  - O_aug^T  += V_aug^T-style matmul(lhsT=V_aug[k128,65], rhs=Em^T[k128,q512]),
                where V_aug's 65th column of ones makes row 64 the softmax
                denominators r[q] for free.

The host then normalizes (divide by r) and transposes back - a pure scaling
epilogue; masked entries are exactly 0 because exp(-1e9) == 0 == exp(s)*0.
bf16 storage of the unnormalized exp keeps rel-err ~5e-3 while halving the
dominant DMA traffic.
"""

import os
import sys

import numpy as np
import ml_dtypes

B, H, S, D = 2, 8, 4096, 64
NPAIRS = 2            # (b*h) pairs per core
N_CORES = 8
QB = 512              # q block (matmul free dim)
NQB = S // QB         # 8 q blocks
KT = 128              # k tile (contraction rows per S^T tile)
NKT = S // KT         # 32 k tiles
CH = 2                # k tiles per PSUM chunk (exp/mask granularity)
NCH = NKT // CH       # 16 chunks per strip
VA = D + 1            # V columns + ones column

BF16 = ml_dtypes.bfloat16

_cache = {}
last_exec_time_ns = None
last_profile = None


def _install_axon_ntff_hook():
    """Provide the missing antenv.axon_hooks module (ctypes NTFF profiling)."""
    import contextlib
    import ctypes
    import types

    if "antenv.axon_hooks" in sys.modules:
        return
    so_path = "/opt/axon/libaxon_pjrt.so"
    try:
        lib = ctypes.CDLL(so_path)
        lib.axon_start_nrt_profile.argtypes = [
            ctypes.POINTER(ctypes.c_int64),
            ctypes.c_size_t,
        ]
        lib.axon_start_nrt_profile.restype = ctypes.c_int64
        lib.axon_stop_nrt_profile.argtypes = [ctypes.c_char_p]
        lib.axon_stop_nrt_profile.restype = ctypes.c_int64
    except (OSError, AttributeError):
        return

    @contextlib.contextmanager
    def _hook(output_dir, device_ids=None):
        import jax

        jax.devices()
        rc = lib.axon_start_nrt_profile(None, 0)
        if rc != 0:
            raise RuntimeError(f"axon_start_nrt_profile rc={rc}")
        try:
            yield
        finally:
            n = lib.axon_stop_nrt_profile(str(output_dir).encode())
            print(f"ntff profile: {n} file(s) -> {output_dir}", file=sys.stderr)

    mod = types.ModuleType("antenv.axon_hooks")
    mod.get_axon_ntff_profile_hook = lambda: _hook
    mod.set_axon_ntff_profile_hook = lambda h: None
    import antenv

    antenv.axon_hooks = mod
    sys.modules["antenv.axon_hooks"] = mod

    from concourse import bass_utils

    bass_utils.upload_artifacts = lambda tmpdir: f"file://{tmpdir}"


def _build():
    import concourse.tile as tile
    from concourse import bacc, mybir

    bf = mybir.dt.bfloat16
    f32 = mybir.dt.float32

    nc = bacc.Bacc("TRN2", target_bir_lowering=False, debug=False,
                   num_devices=N_CORES)

    # qt: Q^T duplicated into both row-halves [128, S] so the two row-packed
    # S^T matmuls (row groups 0-63 / 64-127) each stream their own rhs copy.
    # kt: K^T packed pairwise: partitions 0-63 hold even k-tiles, 64-127 odd.
    qt = nc.declare_dram_parameter("qt", [NPAIRS, 2 * D, S], bf, isOutput=False)
    kt = nc.declare_dram_parameter("kt", [NPAIRS, 2 * D, S // 2], bf, isOutput=False)
    va = nc.declare_dram_parameter("va", [NPAIRS, KT, NKT, VA], bf, isOutput=False)
    mt = nc.declare_dram_parameter("mt", [NKT, KT, S], bf, isOutput=False)
    eout = nc.declare_dram_parameter("eout", [NPAIRS, NKT, KT, S], bf, isOutput=True)
    oaug = nc.declare_dram_parameter("oaug", [NPAIRS, VA, S], f32, isOutput=True)

    mt_r = mt.rearrange("t p q -> p t q")
    eout_r = eout.rearrange("v t p q -> v p t q")

    with tile.TileContext(nc) as tc:
        with (
            tc.tile_pool(name="const", bufs=1) as cpool,
            tc.tile_pool(name="mask", bufs=2) as mpool,
            tc.tile_pool(name="spsum", bufs=3, space="PSUM") as spool,
            tc.tile_pool(name="opsum", bufs=2, space="PSUM") as opool,
            tc.tile_pool(name="echunk", bufs=6) as epool,
            tc.tile_pool(name="emchunk", bufs=20) as empool,
            tc.tile_pool(name="osb", bufs=2) as ospool,
        ):
            qt_sb, kt_sb, va_sb = [], [], []
            for p in range(NPAIRS):
                qts = cpool.tile([2 * D, S], bf, tag=f"qt{p}")
                nc.sync.dma_start(out=qts[:], in_=qt[p])
                qt_sb.append(qts)
                kts = cpool.tile([2 * D, S // 2], bf, tag=f"kt{p}")
                nc.sync.dma_start(out=kts[:], in_=kt[p])
                kt_sb.append(kts)
                vas = cpool.tile([KT, NKT, VA], bf, tag=f"va{p}")
                nc.sync.dma_start(out=vas[:], in_=va[p])
                va_sb.append(vas)

            out_engines = [nc.sync, nc.scalar]
            for qb in range(NQB):
                qsl = slice(qb * QB, (qb + 1) * QB)
                m_strip = mpool.tile([KT, NKT, QB], bf, tag="mstrip")
                nc.sync.dma_start(out=m_strip[:], in_=mt_r[:, :, qsl])

                for p in range(NPAIRS):
                    o_ps = opool.tile([VA, QB], f32, tag="ops")
                    em_chunks = [None] * NCH
                    LAG = 4

                    def emit_pv(c, o_ps=o_ps, em_chunks=em_chunks, p=p):
                        for j in range(CH):
                            k_tile = c * CH + j
                            nc.tensor.matmul(
                                o_ps[:],
                                lhsT=va_sb[p][:, k_tile, :],
                                rhs=em_chunks[c][:, j, :],
                                start=(k_tile == 0),
                                stop=(k_tile == NKT - 1),
                            )

                    for c in range(NCH):
                        ksl = slice(c * KT, (c + 1) * KT)
                        s_ps = spool.tile([KT, CH, QB], f32, tag="sps")
                        # Two k-tiles computed concurrently in PE row groups
                        # 0-63 / 64-127 (contraction is only D=64).
                        nc.tensor.matmul(
                            s_ps[:, 0, :],
                            lhsT=kt_sb[p][0:D, ksl],
                            rhs=qt_sb[p][0:D, qsl],
                            start=True, stop=True,
                            tile_position=(0, 0),
                        )
                        nc.tensor.matmul(
                            s_ps[:, 1, :],
                            lhsT=kt_sb[p][D:2 * D, ksl],
                            rhs=qt_sb[p][D:2 * D, qsl],
                            start=True, stop=True,
                            tile_position=(64, 0),
                        )
                        e_ch = epool.tile([KT, CH, QB], bf, tag="ech")
                        nc.scalar.activation(
                            e_ch[:], s_ps[:],
                            mybir.ActivationFunctionType.Exp,
                        )
                        em_ch = empool.tile([KT, CH, QB], bf, tag="emch")
                        nc.vector.tensor_tensor(
                            em_ch[:], e_ch[:],
                            m_strip[:, c * CH:(c + 1) * CH, :],
                            mybir.AluOpType.mult,
                        )
                        em_chunks[c] = em_ch
                        # PV matmuls lag LAG chunks behind so the PE never
                        # stalls waiting on the exp->mask chain of chunk c.
                        if c >= LAG:
                            emit_pv(c - LAG)
                        eng = out_engines[c % len(out_engines)]
                        eng.dma_start(
                            out=eout_r[p, :, c * CH:(c + 1) * CH, qsl],
                            in_=em_ch[:],
                        )
                    for c in range(NCH - LAG, NCH):
                        emit_pv(c)
                    o_sb = ospool.tile([VA, QB], f32, tag="osb")
                    nc.vector.tensor_copy(o_sb[:], o_ps[:])
                    nc.sync.dma_start(out=oaug[p, :, qsl], in_=o_sb[:])

    nc.compile()
    return nc


def _get_nc():
    if "nc" not in _cache:
        _install_axon_ntff_hook()
        _cache["nc"] = _build()
    return _cache["nc"]


def kernel(query, key, value, mask):
    global last_exec_time_ns, last_profile
    from concourse.bass_utils import run_bass_kernel_spmd

    nc = _get_nc()

    q = np.asarray(query, dtype=np.float32).reshape(B * H, S, D)
    k = np.asarray(key, dtype=np.float32).reshape(B * H, S, D)
    v = np.asarray(value, dtype=np.float32).reshape(B * H, S, D)
    m = np.asarray(mask)

    # Host prep: transpose Q/K to [d, s] (fold the 1/sqrt(D) scale into Q),
    # append the ones column to V and pack partition-major, transpose mask.
    # Q^T is duplicated into rows 64-127 and K^T packed pairwise (even k-tiles
    # in rows 0-63, odd in 64-127) for the row-packed concurrent matmuls.
    qt_t = np.ascontiguousarray(
        (q * (1.0 / np.sqrt(D))).transpose(0, 2, 1)).astype(BF16)
    qt_full = np.concatenate([qt_t, qt_t], axis=1)            # [bh, 128, S]
    kt_t = k.transpose(0, 2, 1)                               # [bh, 64, S]
    kt_full = np.ascontiguousarray(
        kt_t.reshape(B * H, D, NKT // 2, 2, KT)
        .transpose(0, 3, 1, 2, 4)
        .reshape(B * H, 2 * D, S // 2)).astype(BF16)
    v_aug = np.concatenate(
        [v, np.ones((B * H, S, 1), np.float32)], axis=2)      # [bh, S, 65]
    va_full = np.ascontiguousarray(
        v_aug.reshape(B * H, NKT, KT, VA).transpose(0, 2, 1, 3)).astype(BF16)
    mt_host = np.ascontiguousarray(m.T.astype(np.float32)).astype(BF16)
    mt_host = mt_host.reshape(NKT, KT, S)

    in_maps = []
    for c in range(N_CORES):
        sl = slice(NPAIRS * c, NPAIRS * (c + 1))
        in_maps.append({
            "qt": qt_full[sl],
            "kt": kt_full[sl],
            "va": va_full[sl],
            "mt": mt_host,
        })

    trace = bool(int(os.environ.get("BASS_KERNEL_TRACE", "0")))
    res = run_bass_kernel_spmd(nc, in_maps, core_ids=list(range(N_CORES)),
                               trace=trace)
    last_exec_time_ns = res.exec_time_ns
    last_profile = res.profile_json

    out = np.empty((B * H, S, D), np.float32)
    attn = np.empty((B * H, S, S), np.float32)
    for c in range(N_CORES):
        eo = res.results[c]["eout"]     # [NPAIRS, NKT, KT, S] bf16 (k-major)
        oa = res.results[c]["oaug"]     # [NPAIRS, VA, S] f32
        for i in range(NPAIRS):
            bh = NPAIRS * c + i
            r = oa[i, D, :].astype(np.float32)    # softmax denominators [q]
            r = np.where(r == 0.0, 1.0, r)
            inv_r = (1.0 / r).astype(np.float32)
            et = eo[i].reshape(S, S).astype(np.float32)       # [k, q]
            attn[bh] = (et * inv_r[None, :]).T
            out[bh] = (oa[i, :D, :] * inv_r[None, :]).T
    output = out.reshape(B, H, S, D)
    attention = attn.reshape(B, H, S, S)
    return (output, attention)


# revision 34
# speedup vs baseline: 2.3898x; 1.1831x over previous
"""Masked attention (B=2,H=8,S=4096,D=64) on 8 Trainium2 NeuronCores.

Strategy (head-parallel, no collectives): the 16 (b,h) pairs shard 2-per-core.
The attention matrix (the reference's second output, [B,H,S,S] f32 ~= 1 GiB) is
what makes this memory-bound, so the device computes and ships everything in
the TRANSPOSED orientation S^T[k,q] = K @ Q^T:

  - S^T tile  = matmul(lhsT=K^T[d,k128], rhs=Q^T[d,q512])   (Q pre-scaled 1/8)
  - E^T       = exp(S^T)      on ScalarE (PSUM -> SBUF bf16, one pass)
  - Em^T      = E^T * mask^T  on VectorE (bf16 2x mode)     -> DMA out (bf16)
  - O_aug^T  += V_aug^T-style matmul(lhsT=V_aug[k128,65], rhs=Em^T[k128,q512]),
                where V_aug's 65th column of ones makes row 64 the softmax
                denominators r[q] for free.

The host then normalizes (divide by r) and transposes back - a pure scaling
epilogue; masked entries are exactly 0 because exp(-1e9) == 0 == exp(s)*0.
bf16 storage of the unnormalized exp keeps rel-err ~5e-3 while halving the
dominant DMA traffic.
"""

import os
import sys

import numpy as np
import ml_dtypes

B, H, S, D = 2, 8, 4096, 64
NPAIRS = 2            # (b*h) pairs per core
N_CORES = 8
QB = 512              # q block (matmul free dim)
NQB = S // QB         # 8 q blocks
KT = 128              # k tile (contraction rows per S^T tile)
NKT = S // KT         # 32 k tiles
CH = 2                # k tiles per PSUM chunk (exp/mask granularity)
NCH = NKT // CH       # 16 chunks per strip
VA = D + 1            # V columns + ones column

BF16 = ml_dtypes.bfloat16

_cache = {}
last_exec_time_ns = None
last_profile = None


def _install_axon_ntff_hook():
    """Provide the missing antenv.axon_hooks module (ctypes NTFF profiling)."""
    import contextlib
    import ctypes
    import types

    if "antenv.axon_hooks" in sys.modules:
        return
    so_path = "/opt/axon/libaxon_pjrt.so"
    try:
        lib = ctypes.CDLL(so_path)
        lib.axon_start_nrt_profile.argtypes = [
            ctypes.POINTER(ctypes.c_int64),
            ctypes.c_size_t,
        ]
        lib.axon_start_nrt_profile.restype = ctypes.c_int64
        lib.axon_stop_nrt_profile.argtypes = [ctypes.c_char_p]
        lib.axon_stop_nrt_profile.restype = ctypes.c_int64
    except (OSError, AttributeError):
        return

    @contextlib.contextmanager
    def _hook(output_dir, device_ids=None):
        import jax

        jax.devices()
        rc = lib.axon_start_nrt_profile(None, 0)
        if rc != 0:
            raise RuntimeError(f"axon_start_nrt_profile rc={rc}")
        try:
            yield
        finally:
            n = lib.axon_stop_nrt_profile(str(output_dir).encode())
            print(f"ntff profile: {n} file(s) -> {output_dir}", file=sys.stderr)

    mod = types.ModuleType("antenv.axon_hooks")
    mod.get_axon_ntff_profile_hook = lambda: _hook
    mod.set_axon_ntff_profile_hook = lambda h: None
    import antenv

    antenv.axon_hooks = mod
    sys.modules["antenv.axon_hooks"] = mod

    from concourse import bass_utils

    bass_utils.upload_artifacts = lambda tmpdir: f"file://{tmpdir}"


def _build():
    import concourse.tile as tile
    from concourse import bacc, mybir

    bf = mybir.dt.bfloat16
    f32 = mybir.dt.float32

    nc = bacc.Bacc("TRN2", target_bir_lowering=False, debug=False,
                   num_devices=N_CORES)

    # qt: Q^T duplicated into both row-halves [128, S] so the two row-packed
    # S^T matmuls (row groups 0-63 / 64-127) each stream their own rhs copy.
    # kt: K^T packed pairwise: partitions 0-63 hold even k-tiles, 64-127 odd.
    qt = nc.declare_dram_parameter("qt", [NPAIRS, 2 * D, S], bf, isOutput=False)
    kt = nc.declare_dram_parameter("kt", [NPAIRS, 2 * D, S // 2], bf, isOutput=False)
    va = nc.declare_dram_parameter("va", [NPAIRS, KT, NKT, VA], bf, isOutput=False)
    # mt: mask^T pre-permuted per q-strip so each strip DMA is one fully
    # contiguous 32KB run per partition.  eout: chunk-contiguous (2KB runs);
    # the host reassembles [k,q] order during the normalize epilogue.
    mt = nc.declare_dram_parameter("mt", [NQB, KT, NKT, QB], bf, isOutput=False)
    eout = nc.declare_dram_parameter(
        "eout", [NPAIRS * NQB * NCH, KT, CH, QB], bf, isOutput=True)
    oaug = nc.declare_dram_parameter("oaug", [NPAIRS, VA, S], f32, isOutput=True)

    with tile.TileContext(nc) as tc:
        with (
            tc.tile_pool(name="const", bufs=1) as cpool,
            tc.tile_pool(name="mask", bufs=2) as mpool,
            tc.tile_pool(name="spsum", bufs=3, space="PSUM") as spool,
            tc.tile_pool(name="opsum", bufs=2, space="PSUM") as opool,
            tc.tile_pool(name="echunk", bufs=6) as epool,
            tc.tile_pool(name="emchunk", bufs=20) as empool,
            tc.tile_pool(name="osb", bufs=2) as ospool,
        ):
            qt_sb, kt_sb, va_sb = [], [], []
            for p in range(NPAIRS):
                qts = cpool.tile([2 * D, S], bf, tag=f"qt{p}")
                nc.sync.dma_start(out=qts[:], in_=qt[p])
                qt_sb.append(qts)
                kts = cpool.tile([2 * D, S // 2], bf, tag=f"kt{p}")
                nc.sync.dma_start(out=kts[:], in_=kt[p])
                kt_sb.append(kts)
                vas = cpool.tile([KT, NKT, VA], bf, tag=f"va{p}")
                nc.sync.dma_start(out=vas[:], in_=va[p])
                va_sb.append(vas)

            for qb in range(NQB):
                qsl = slice(qb * QB, (qb + 1) * QB)
                m_strip = mpool.tile([KT, NKT, QB], bf, tag="mstrip")
                nc.sync.dma_start(out=m_strip[:], in_=mt[qb])

                for p in range(NPAIRS):
                    o_ps = opool.tile([VA, QB], f32, tag="ops")
                    em_chunks = [None] * NCH
                    LAG = 4

                    def emit_pv(c, o_ps=o_ps, em_chunks=em_chunks, p=p):
                        for j in range(CH):
                            k_tile = c * CH + j
                            nc.tensor.matmul(
                                o_ps[:],
                                lhsT=va_sb[p][:, k_tile, :],
                                rhs=em_chunks[c][:, j, :],
                                start=(k_tile == 0),
                                stop=(k_tile == NKT - 1),
                            )

                    for c in range(NCH):
                        ksl = slice(c * KT, (c + 1) * KT)
                        s_ps = spool.tile([KT, CH, QB], f32, tag="sps")
                        # Two k-tiles computed concurrently in PE row groups
                        # 0-63 / 64-127 (contraction is only D=64).
                        nc.tensor.matmul(
                            s_ps[:, 0, :],
                            lhsT=kt_sb[p][0:D, ksl],
                            rhs=qt_sb[p][0:D, qsl],
                            start=True, stop=True,
                            tile_position=(0, 0),
                        )
                        nc.tensor.matmul(
                            s_ps[:, 1, :],
                            lhsT=kt_sb[p][D:2 * D, ksl],
                            rhs=qt_sb[p][D:2 * D, qsl],
                            start=True, stop=True,
                            tile_position=(64, 0),
                        )
                        e_ch = epool.tile([KT, CH, QB], bf, tag="ech")
                        nc.scalar.activation(
                            e_ch[:], s_ps[:],
                            mybir.ActivationFunctionType.Exp,
                        )
                        em_ch = empool.tile([KT, CH, QB], bf, tag="emch")
                        nc.vector.tensor_tensor(
                            em_ch[:], e_ch[:],
                            m_strip[:, c * CH:(c + 1) * CH, :],
                            mybir.AluOpType.mult,
                        )
                        em_chunks[c] = em_ch
                        # PV matmuls lag LAG chunks behind so the PE never
                        # stalls waiting on the exp->mask chain of chunk c.
                        if c >= LAG:
                            emit_pv(c - LAG)
                        nc.sync.dma_start(
                            out=eout[(p * NQB + qb) * NCH + c],
                            in_=em_ch[:],
                        )
                    for c in range(NCH - LAG, NCH):
                        emit_pv(c)
                    o_sb = ospool.tile([VA, QB], f32, tag="osb")
                    nc.vector.tensor_copy(o_sb[:], o_ps[:])
                    nc.sync.dma_start(out=oaug[p, :, qsl], in_=o_sb[:])

    nc.compile()
    return nc


def _get_nc():
    if "nc" not in _cache:
        _install_axon_ntff_hook()
        _cache["nc"] = _build()
    return _cache["nc"]


def kernel(query, key, value, mask):
    global last_exec_time_ns, last_profile
    from concourse.bass_utils import run_bass_kernel_spmd

    nc = _get_nc()

    q = np.asarray(query, dtype=np.float32).reshape(B * H, S, D)
    k = np.asarray(key, dtype=np.float32).reshape(B * H, S, D)
    v = np.asarray(value, dtype=np.float32).reshape(B * H, S, D)
    m = np.asarray(mask)

    # Host prep: transpose Q/K to [d, s] (fold the 1/sqrt(D) scale into Q),
    # append the ones column to V and pack partition-major, transpose mask.
    # Q^T is duplicated into rows 64-127 and K^T packed pairwise (even k-tiles
    # in rows 0-63, odd in 64-127) for the row-packed concurrent matmuls.
    qt_t = np.ascontiguousarray(
        (q * (1.0 / np.sqrt(D))).transpose(0, 2, 1)).astype(BF16)
    qt_full = np.concatenate([qt_t, qt_t], axis=1)            # [bh, 128, S]
    kt_t = k.transpose(0, 2, 1)                               # [bh, 64, S]
    kt_full = np.ascontiguousarray(
        kt_t.reshape(B * H, D, NKT // 2, 2, KT)
        .transpose(0, 3, 1, 2, 4)
        .reshape(B * H, 2 * D, S // 2)).astype(BF16)
    v_aug = np.concatenate(
        [v, np.ones((B * H, S, 1), np.float32)], axis=2)      # [bh, S, 65]
    va_full = np.ascontiguousarray(
        v_aug.reshape(B * H, NKT, KT, VA).transpose(0, 2, 1, 3)).astype(BF16)
    mt_host = np.ascontiguousarray(
        m.T.astype(np.float32).reshape(NKT, KT, NQB, QB).transpose(2, 1, 0, 3)
    ).astype(BF16)                                            # [qb, p, t, q']

    in_maps = []
    for c in range(N_CORES):
        sl = slice(NPAIRS * c, NPAIRS * (c + 1))
        in_maps.append({
            "qt": qt_full[sl],
            "kt": kt_full[sl],
            "va": va_full[sl],
            "mt": mt_host,
        })

    trace = bool(int(os.environ.get("BASS_KERNEL_TRACE", "0")))
    res = run_bass_kernel_spmd(nc, in_maps, core_ids=list(range(N_CORES)),
                               trace=trace)
    last_exec_time_ns = res.exec_time_ns
    last_profile = res.profile_json

    out = np.empty((B * H, S, D), np.float32)
    attn = np.empty((B * H, S, S), np.float32)
    for c in range(N_CORES):
        # eout chunks: [pair, qb, c, pp, j, q'] with k = (c*CH+j)*KT + pp
        eo = res.results[c]["eout"].reshape(NPAIRS, NQB, NCH, KT, CH, QB)
        oa = res.results[c]["oaug"]     # [NPAIRS, VA, S] f32
        for i in range(NPAIRS):
            bh = NPAIRS * c + i
            r = oa[i, D, :].astype(np.float32)    # softmax denominators [q]
            r = np.where(r == 0.0, 1.0, r)
            inv_r = (1.0 / r).astype(np.float32)
            x = eo[i].astype(np.float32)
            x *= inv_r.reshape(NQB, 1, 1, 1, QB)
            attn[bh] = x.transpose(0, 4, 1, 3, 2).reshape(S, S)
            out[bh] = (oa[i, :D, :] * inv_r[None, :]).T
    output = out.reshape(B, H, S, D)
    attention = attn.reshape(B, H, S, S)
    return (output, attention)
